# revision 8
# baseline (speedup 1.0000x reference)
"""Masked multi-head attention on 8 TRN2 NeuronCores.

Sharding: 8 cores = 2 batches x 4 head-groups (4 heads of 64 dims each).
Each core: Q^T/K^T projections (head-dim on partitions) with biases folded
in as K=1 matmuls, V computed directly in key-major layout [keys, vdim]
(xt tiles stationary) into v_aug blocks [64 vd | ones] per head so the PV
matmul yields numerator rows 0-63 and the softmax denominator in row 64 of
one PSUM accumulation. Scores S^T = K^T.T @ Q^T per 128-key tile; the two
heads of a group run as concurrent row-packed K=64 matmuls (tile_position
auto-derived from base partitions). Above-diagonal tiles skipped, diagonal
tiles column-restricted and masked post-exp by a 0/1 triangle multiply.
exp on ScalarE (no max subtraction: scores ~N(0,1)). Scores are emitted
one key-tile ahead so ScalarE (the bottleneck at ~60% of runtime) never
starves; group-1 projections are drained just-in-time into the group-0
attention loop. Output is attn^T bf16; the host transposes/concats/casts.
"""
import threading
from contextlib import ExitStack

import ml_dtypes
import numpy as np

import concourse.bass as bass
import concourse.tile as tile
from concourse import bacc, mybir
from concourse.bass_utils import run_bass_kernel_spmd

F32 = mybir.dt.float32
MMDT = mybir.dt.bfloat16
NPDT = ml_dtypes.bfloat16
EXP = mybir.ActivationFunctionType.Exp

B, T, C = 2, 2048, 1024
H, DH = 16, 64
HPC = 4            # heads per core
RPC = HPC * DH     # 256 output channels per core
NCT = C // 128     # 8 contraction tiles
NQC = T // 512     # 4 query chunks
NKT = T // 128     # 16 key tiles
N_WARM = 12        # PE warmup matmuls during the DMA head


class _Filler:
    """Ordered stream of (tag, generator) emission blocks.

    drain_through(tag) finishes every block up to and including `tag`;
    step(n) advances n yields from the current position (opportunistic
    interleave of projection work into the attention loop).
    """

    def __init__(self, blocks):
        self.blocks = [(t, iter(g)) for t, g in blocks]
        self.i = 0

    def step(self, n=1):
        for _ in range(n):
            while self.i < len(self.blocks):
                try:
                    next(self.blocks[self.i][1])
                    break
                except StopIteration:
                    self.i += 1
            else:
                return

    def drain_through(self, tag):
        idx = None
        for j in range(self.i, len(self.blocks)):
            if self.blocks[j][0] == tag:
                idx = j
                break
        if idx is None:
            return
        while self.i <= idx:
            try:
                next(self.blocks[self.i][1])
            except StopIteration:
                self.i += 1

    def drain_all(self):
        while self.i < len(self.blocks):
            try:
                next(self.blocks[self.i][1])
            except StopIteration:
                self.i += 1


def _build():
    nc = bacc.Bacc("TRN2", target_bir_lowering=False, debug=False)
    xt = nc.dram_tensor("xt", [C, T], MMDT, kind="ExternalInput").ap()
    wq = nc.dram_tensor("wq", [C, RPC], MMDT, kind="ExternalInput").ap()
    wk = nc.dram_tensor("wk", [C, RPC], MMDT, kind="ExternalInput").ap()
    wv = nc.dram_tensor("wv", [C, RPC], MMDT, kind="ExternalInput").ap()
    bq = nc.dram_tensor("bq", [1, RPC], MMDT, kind="ExternalInput").ap()
    bk = nc.dram_tensor("bk", [1, RPC], MMDT, kind="ExternalInput").ap()
    bv = nc.dram_tensor("bv", [1, RPC], MMDT, kind="ExternalInput").ap()
    tri = nc.dram_tensor("tri", [128, 256], MMDT, kind="ExternalInput").ap()
    ones = nc.dram_tensor("ones", [1, 512], MMDT, kind="ExternalInput").ap()
    ot = nc.dram_tensor("ot", [RPC, T], MMDT, kind="ExternalOutput").ap()

    with tile.TileContext(nc) as tc, ExitStack() as ctx:
        per = ctx.enter_context(tc.tile_pool(name="per", bufs=1))
        wrk = ctx.enter_context(tc.tile_pool(name="wrk", bufs=1))
        ps = ctx.enter_context(tc.tile_pool(name="ps", bufs=1, space="PSUM"))

        # ---- persistent SBUF tiles ----
        wq_s = per.tile([128, NCT, RPC], MMDT, tag="wq")
        wk_s = per.tile([128, NCT, RPC], MMDT, tag="wk")
        wv_s = per.tile([128, NCT, RPC], MMDT, tag="wv")
        bq_s = per.tile([1, RPC], MMDT, tag="bq")
        bk_s = per.tile([1, RPC], MMDT, tag="bk")
        bv_s = per.tile([1, RPC], MMDT, tag="bv")
        ones_s = per.tile([1, 512], MMDT, tag="ones")
        tri_s = per.tile([128, 2, 128], MMDT, tag="tri")
        xt_s = per.tile([128, NCT, T], MMDT, tag="xt")
        qt_s = per.tile([128, 2, T], MMDT, tag="qt")
        kt_s = per.tile([128, 2, T], MMDT, tag="kt")
        # per (key tile, group): [hA 64 | one | hB 64 | one]
        v_aug = per.tile([128, NKT, 2, 130], MMDT, tag="vaug")
        junk = per.tile([128, 512], MMDT, tag="junk")

        nc.gpsimd.memset(junk[:], 0.0)
        nc.gpsimd.memset(v_aug[:, :, :, 64:65], 1.0)
        nc.gpsimd.memset(v_aug[:, :, :, 129:130], 1.0)

        # ---- DMA loads (order = availability order) ----
        nc.sync.dma_start(bq_s[:], bq[:])
        nc.sync.dma_start(bk_s[:], bk[:])
        nc.sync.dma_start(bv_s[:], bv[:])
        nc.sync.dma_start(ones_s[:], ones[:])
        nc.sync.dma_start(tri_s[:], tri.rearrange("p (a b) -> p a b", a=2))
        xt_r = xt.rearrange("(c p) t -> p c t", p=128)
        nc.sync.dma_start(wq_s[:], wq.rearrange("(c p) m -> p c m", p=128))
        nc.sync.dma_start(xt_s[:, :, 0:512], xt_r[:, :, 0:512])
        nc.sync.dma_start(wk_s[:], wk.rearrange("(c p) m -> p c m", p=128))
        nc.sync.dma_start(wv_s[:], wv.rearrange("(c p) m -> p c m", p=128))
        for cc in (1, 2, 3):
            nc.sync.dma_start(
                xt_s[:, :, 512 * cc:512 * (cc + 1)],
                xt_r[:, :, 512 * cc:512 * (cc + 1)])

        # warm the exp table while DMAs run
        warm = wrk.tile([1, 8], MMDT, tag="warm")
        nc.scalar.activation(warm[:], junk[0:1, 0:8], EXP)
        # PE warmup stream (junk matmuls, no DMA deps) to lift HAM to 8/8
        for i in range(N_WARM):
            jt = ps.tile([128, 2, 512], F32, tag="s2", bufs=2, name=f"jt{i}")
            nc.tensor.matmul(jt[:, 0, :], junk[:, 0:128], junk[:],
                             start=True, stop=True)

        # ---- projection emitters ----
        def qk_steps(w_s, b_s, o_s, g, c):
            pq = ps.tile([128, 512], F32, tag="pp", bufs=2,
                         name=f"pq_{o_s.tensor.name}_{g}_{c}")
            nc.tensor.matmul(pq[:], b_s[0:1, 128 * g:128 * (g + 1)],
                             ones_s[:], start=True, stop=False)
            yield
            for ct in range(NCT):
                nc.tensor.matmul(
                    pq[:], w_s[:, ct, 128 * g:128 * (g + 1)],
                    xt_s[:, ct, 512 * c:512 * (c + 1)],
                    start=False, stop=(ct == NCT - 1))
                yield
            nc.vector.tensor_copy(o_s[:, g, 512 * c:512 * (c + 1)], pq[:])
            yield

        def v_steps(kt):
            vp = ps.tile([128, 512], F32, tag="pp", bufs=2, name=f"vp{kt}")
            vps = vp[:, 0:RPC]
            nc.tensor.matmul(vps, ones_s[0:1, 0:128], bv_s[:],
                             start=True, stop=False)
            yield
            for ct in range(NCT):
                nc.tensor.matmul(
                    vps, xt_s[:, ct, 128 * kt:128 * (kt + 1)],
                    wv_s[:, ct, :], start=False, stop=(ct == NCT - 1))
                yield
            for g in range(2):
                nc.vector.tensor_copy(
                    v_aug[:, kt, g, 0:64], vp[:, 128 * g:128 * g + 64])
                yield
                nc.vector.tensor_copy(
                    v_aug[:, kt, g, 65:129], vp[:, 128 * g + 64:128 * g + 128])
                yield

        def chain(*gens):
            for gg in gens:
                yield from gg

        def run(gen):
            for _ in gen:
                pass

        # pre-attention: only what the first scores need (V is drained JIT
        # before the first PV, underneath the first scores+exps)
        run(qk_steps(wq_s, bq_s, qt_s, 0, 0))
        run(qk_steps(wk_s, bk_s, kt_s, 0, 0))

        # remaining projections, in need-order, drained JIT + opportunistically
        blocks = [(f"v{kt}", v_steps(kt)) for kt in range(4)]
        for c in (1, 2, 3):
            blocks.append((f"qk0{c}", chain(
                qk_steps(wq_s, bq_s, qt_s, 0, c),
                qk_steps(wk_s, bk_s, kt_s, 0, c))))
            for kt in range(4 * c, 4 * c + 4):
                blocks.append((f"v{kt}", v_steps(kt)))
        for c in range(NQC):
            blocks.append((f"qk1{c}", chain(
                qk_steps(wq_s, bq_s, qt_s, 1, c),
                qk_steps(wk_s, bk_s, kt_s, 1, c))))
        filler = _Filler(blocks)

        # ---- attention ----
        def emit_S(g, c, kt):
            w0 = max(0, 128 * (kt - 4 * c))
            s2_t = ps.tile([128, 2, 512], F32, tag="s2", bufs=2,
                           name=f"s2_{g}_{c}_{kt}")
            ksl = slice(128 * kt, 128 * (kt + 1))
            qsl = slice(512 * c + w0, 512 * (c + 1))
            nc.tensor.matmul(s2_t[:, 0, w0:512], kt_s[0:64, g, ksl],
                             qt_s[0:64, g, qsl], start=True, stop=True)
            nc.tensor.matmul(s2_t[:, 1, w0:512], kt_s[64:128, g, ksl],
                             qt_s[64:128, g, qsl], start=True, stop=True)
            return s2_t, w0

        def make_norm(g, c, ntA, ntB):
            def norm():
                for h, nt in ((0, ntA), (1, ntB)):
                    dr = wrk.tile([1, 512], MMDT, tag="dr", bufs=2,
                                  name=f"dr_{g}_{c}_{h}")
                    nc.vector.tensor_copy(dr[:], nt[64:65, :])
                    rbp = ps.tile([128, 512], F32, tag="pp", bufs=2,
                                  name=f"rbp_{g}_{c}_{h}")
                    nc.tensor.matmul(rbp[0:64, :], ones_s[0:1, 0:64], dr[:],
                                     start=True, stop=True)
                    rb = wrk.tile([64, 512], F32, tag="rb", bufs=2,
                                  name=f"rb_{g}_{c}_{h}")
                    nc.vector.reciprocal_approx_fast(out=rb[:], in_=rbp[0:64, :])
                    oc = wrk.tile([64, 512], MMDT, tag="oc", bufs=4,
                                  name=f"oc_{g}_{c}_{h}")
                    nc.vector.tensor_mul(oc[:], nt[0:64, :], rb[:])
                    nc.sync.dma_start(
                        ot[64 * (2 * g + h):64 * (2 * g + h) + 64,
                           512 * c:512 * (c + 1)], oc[:])
            return norm

        pending_norm = None
        for g in range(2):
            for c in range(NQC):
                if (g, c) != (0, 0):
                    filler.drain_through(f"qk{g}{c}")
                nkt = 4 * c + 4
                ntA = ps.tile([128, 512], F32, tag="ntA", bufs=1,
                              name=f"ntA{g}{c}")
                ntB = ps.tile([128, 512], F32, tag="ntB", bufs=1,
                              name=f"ntB{g}{c}")
                pend = emit_S(g, c, 0)
                if pending_norm is not None:
                    pending_norm()
                    pending_norm = None
                for kt in range(nkt):
                    s2_t, w0 = pend
                    if kt + 1 < nkt:
                        pend = emit_S(g, c, kt + 1)
                    e2 = wrk.tile([128, 2, 512], MMDT, tag="e2", bufs=3,
                                  name=f"e2_{g}_{c}_{kt}")
                    nc.scalar.activation(e2[:, :, w0:512], s2_t[:, :, w0:512],
                                         EXP)
                    if kt >= 4 * c:
                        nc.vector.tensor_mul(
                            e2[:, :, w0:w0 + 128], e2[:, :, w0:w0 + 128],
                            tri_s[:])
                    filler.step(3)
                    if g == 0:
                        filler.drain_through(f"v{kt}")
                    nc.tensor.matmul(
                        ntA[0:65, w0:512], v_aug[:, kt, g, 0:65],
                        e2[:, 0, w0:512], start=(kt == 0),
                        stop=(kt == nkt - 1), skip_group_check=True)
                    nc.tensor.matmul(
                        ntB[0:65, w0:512], v_aug[:, kt, g, 65:130],
                        e2[:, 1, w0:512], start=(kt == 0),
                        stop=(kt == nkt - 1), skip_group_check=True)
                    filler.step(2)
                pending_norm = make_norm(g, c, ntA, ntB)
        filler.drain_all()
        pending_norm()

    nc.compile()
    return nc


_LOCK = threading.Lock()
_NC = None


def _get_nc():
    global _NC
    with _LOCK:
        if _NC is None:
            _NC = _build()
    return _NC


def _tri_tile():
    p = np.arange(128)[:, None]
    j = np.arange(128)[None, :]
    t = (j >= p).astype(np.float32)
    return np.concatenate([t, t], axis=1).astype(NPDT)


def _shard_inputs(X, Wq, bq, Wk, bk, Wv, bv):
    X = np.asarray(X, dtype=np.float32)
    Wq = np.asarray(Wq, dtype=np.float32)
    Wk = np.asarray(Wk, dtype=np.float32)
    Wv = np.asarray(Wv, dtype=np.float32)
    bq = np.asarray(bq, dtype=np.float32)
    bk = np.asarray(bk, dtype=np.float32)
    bv = np.asarray(bv, dtype=np.float32)
    s = np.float32(1.0 / np.sqrt(DH))
    tri = _tri_tile()
    ones = np.ones((1, 512), dtype=NPDT)
    in_maps = []
    for core in range(8):
        b, gq = divmod(core, 4)
        sl = slice(RPC * gq, RPC * (gq + 1))
        in_maps.append({
            "xt": np.ascontiguousarray(X[b].T).astype(NPDT),
            "wq": np.ascontiguousarray((Wq[sl] * s).T).astype(NPDT),
            "wk": np.ascontiguousarray(Wk[sl].T).astype(NPDT),
            "wv": np.ascontiguousarray(Wv[sl].T).astype(NPDT),
            "bq": (bq[sl] * s).reshape(1, RPC).astype(NPDT),
            "bk": bk[sl].reshape(1, RPC).astype(NPDT),
            "bv": bv[sl].reshape(1, RPC).astype(NPDT),
            "tri": tri,
            "ones": ones,
        })
    return in_maps


def kernel(X, Wq, bq, Wk, bk, Wv, bv):
    nc = _get_nc()
    in_maps = _shard_inputs(X, Wq, bq, Wk, bk, Wv, bv)
    res = run_bass_kernel_spmd(nc, in_maps, core_ids=list(range(8)))
    out = np.empty((B, T, C), dtype=np.float32)
    for core in range(8):
        b, gq = divmod(core, 4)
        out[b, :, RPC * gq:RPC * (gq + 1)] = (
            res.results[core]["ot"].astype(np.float32).T)
    return out
